# revision 17
# baseline (speedup 1.0000x reference)
"""Trainium2 Bass kernel for MultiInputModel (gnn_message_passing).

Math:
    gathered = state[:, idx]                       # [B, N, E]
    y   = tanh(einsum('bne,ne->bn', gathered, W) + b)   # [B, N]
    out = 500 * sigmoid(y @ Wf.T)                  # [B, A]

The gather + per-node linear is folded on the host into one dense matrix
A[c, n] = sum_e W[n, e] * [idx[n, e] == c], so the device computes two dense
matmuls with fused activations:
    yT  = tanh(A.T @ stateT + b)        # [N, Bc]  (node dim on partitions)
    out = 500 * sigmoid(yT.T @ WfT)     # [Bc, A]  (batch dim on partitions)

Matmul operands are fp16 (1 PE cycle/row vs 4 for fp32; half the input DMA
bytes); accumulation is fp32 in PSUM and the activations/output stay fp32.

Sharding: batch 8192 -> 8 cores x 1024 rows; A / b / WfT replicated.

Input layout: everything the matmuls read is packed on the host into two
128-partition fp16 DRAM tensors so the whole input side is 3 large DMAs:
  pk1 [128, 3840]: stateT as 3 c-chunks [128,1024] + A as 3 c-chunks [128,256]
  pk2 [128, 8192]: WfT [256,4096] as [k0h0|k1h0|k0h1|k1h1] 2048-col groups
                   (h = 2048-wide half of the action dim), DMA'd per half.
"""

import numpy as np

import concourse.bass as bass
import concourse.tile as tile
from concourse import bacc, mybir
from concourse.bass_utils import run_bass_kernel_spmd

N_CORES = 8
BATCH = 8192
B_CORE = BATCH // N_CORES  # 1024
STATE_DIM = 322
N_NODES = 256
ACTION = 4096

F32 = mybir.dt.float32
F16 = mybir.dt.float16
# contraction (state-dim) chunks: 322 = 128 + 128 + 66
C_CHUNKS = [(0, 128), (128, 128), (256, 66)]
AF = mybir.ActivationFunctionType

HB = 2048  # half-block: ACT/DVE/psum granularity (4 PSUM banks)
PK1_COLS = 3 * B_CORE + 3 * N_NODES  # 3840
PK2_COLS = 2 * ACTION  # 8192


def _build_program() -> bass.Bass:
    # Bacc (not raw Bass): its compile pipeline splits multi-sem waits
    # (move_matmul_waits_to_ldweights / generate_event_semaphores) that the
    # TRN2 ISA requires — raw Bass programs fail walrus codegen on any
    # matmul with >1 semaphore wait.
    nc = bacc.Bacc("TRN2", target_bir_lowering=False, debug=False,
                   num_devices=N_CORES)

    pk1 = nc.dram_tensor("pk1", [128, PK1_COLS], F16, kind="ExternalInput")
    pk2 = nc.dram_tensor("pk2", [128, PK2_COLS], F16, kind="ExternalInput")
    bvec = nc.dram_tensor("bvec", [128, 2], F32, kind="ExternalInput")
    out = nc.dram_tensor("out", [B_CORE, ACTION], F32, kind="ExternalOutput")

    with tile.TileContext(nc) as tc:
        with (
            tc.tile_pool(name="persist", bufs=1) as pp,
            tc.tile_pool(name="sig", bufs=3) as sigp,
            tc.tile_pool(name="obuf", bufs=4) as op,
            tc.tile_pool(name="ps", bufs=2, space="PSUM") as pso,
        ):
            # warm the sigmoid ACT table while input DMAs stream: every
            # activation below is Sigmoid (tanh(x) = 2*sigmoid(2x) - 1), so
            # the one table load happens here, off the critical path.
            warm = pp.tile([128, 1], F32, tag="warm")
            nc.vector.memset(warm, 0.0)
            nc.scalar.activation(out=warm, in_=warm, func=AF.Sigmoid)

            # ---- input DMAs: bias, phase-A pack per c-chunk, 2x wf halves ----
            CGRP = B_CORE + N_NODES  # one c-chunk group: stateT cols + A cols
            bias_t = pp.tile([128, 2], F32, tag="bias")
            nc.sync.dma_start(out=bias_t, in_=bvec[:, :])
            t1 = pp.tile([128, PK1_COLS], F16, tag="t1")
            for ci in range(3):
                nc.sync.dma_start(
                    out=t1[:, ci * CGRP : (ci + 1) * CGRP],
                    in_=pk1[:, ci * CGRP : (ci + 1) * CGRP],
                )
            t2 = pp.tile([128, PK2_COLS], F16, tag="t2")
            for h in range(2):
                nc.sync.dma_start(
                    out=t2[:, h * ACTION : (h + 1) * ACTION],
                    in_=pk2[:, h * ACTION : (h + 1) * ACTION],
                )

            def s_ap(ci, bsl):  # stateT chunk ci, batch slice
                return t1[:, ci * CGRP : ci * CGRP + B_CORE][:, bsl]

            def a_ap(ci, nsl):  # A chunk ci, node slice
                base = ci * CGRP + B_CORE
                return t1[:, base : base + N_NODES][:, nsl]

            def wf_ap(k, ai):  # WfT k-half, 512-wide action chunk ai
                h, aj = divmod(ai, HB // 512)
                base = h * ACTION + k * HB
                return t2[:, base + aj * 512 : base + (aj + 1) * 512]

            y_sb = [
                pp.tile([128, B_CORE], F16, tag=f"y{k}", name=f"y{k}")
                for k in range(2)
            ]

            # ---- phase A: yT = tanh(A.T @ stateT + b)  [256, B_CORE] ----
            # tanh(x) = 2*sigmoid(2x) - 1: ACT does sigmoid(2x + 2b) (host
            # passes 2b), DVE applies the 2t-1 affine while casting to f16.
            # bj-outer so the low batch columns (which phase B reads first)
            # finish first.
            ps = pso.tile([128, HB], F32, tag="ps", name="ps_a")
            for bj in range(B_CORE // 512):
                for nk in range(2):
                    dst = ps[:, nk * 1024 + bj * 512 : nk * 1024 + (bj + 1) * 512]
                    bsl = slice(bj * 512, (bj + 1) * 512)
                    nsl = slice(nk * 128, (nk + 1) * 128)
                    for ci, (c0, cl) in enumerate(C_CHUNKS):
                        nc.tensor.matmul(
                            dst,
                            lhsT=a_ap(ci, nsl)[:cl],
                            rhs=s_ap(ci, bsl)[:cl],
                            start=(ci == 0),
                            stop=(ci == len(C_CHUNKS) - 1),
                        )
            for bj in range(B_CORE // 512):
                for nk in range(2):
                    t = sigp.tile([128, 512], F32, tag=f"sa{nk}{bj}",
                                  name=f"sa{nk}{bj}")
                    nc.scalar.activation(
                        out=t,
                        in_=ps[:, nk * 1024 + bj * 512 : nk * 1024 + (bj + 1) * 512],
                        func=AF.Sigmoid,
                        bias=bias_t[:, nk : nk + 1],
                        scale=2.0,
                    )
                    nc.vector.tensor_scalar(
                        out=y_sb[nk][:, bj * 512 : (bj + 1) * 512],
                        in0=t,
                        scalar1=2.0,
                        scalar2=-1.0,
                        op0=mybir.AluOpType.mult,
                        op1=mybir.AluOpType.add,
                    )

            # ---- phase B: out = 500 * sigmoid(yT.T @ WfT)  [B_CORE, A] ----
            for bi in range(B_CORE // 128):
                ot = op.tile([128, ACTION], F32, tag="ot")
                for h in range(ACTION // HB):
                    ps = pso.tile([128, HB], F32, tag="ps")
                    for aj in range(HB // 512):
                        ai = h * (HB // 512) + aj
                        for k in range(2):
                            nc.tensor.matmul(
                                ps[:, aj * 512 : (aj + 1) * 512],
                                lhsT=y_sb[k][:, bi * 128 : (bi + 1) * 128],
                                rhs=wf_ap(k, ai),
                                start=(k == 0),
                                stop=(k == 1),
                            )
                    sg = sigp.tile([128, HB], F32, tag="sg")
                    nc.scalar.activation(out=sg, in_=ps, func=AF.Sigmoid)
                    nc.vector.tensor_scalar_mul(
                        ot[:, h * HB : (h + 1) * HB], sg, 500.0
                    )
                    nc.sync.dma_start(
                        out=out[bi * 128 : (bi + 1) * 128, h * HB : (h + 1) * HB],
                        in_=ot[:, h * HB : (h + 1) * HB],
                    )

    nc.finalize()  # Bacc.finalize -> compile(): reg alloc, wait splitting, ...
    return nc


def _prepare_in_maps(state, W, b, Wf, idx):
    state = np.asarray(state, dtype=np.float32)
    W = np.asarray(W, dtype=np.float32)
    b = np.asarray(b, dtype=np.float32)
    Wf = np.asarray(Wf, dtype=np.float32)
    idx = np.asarray(idx)

    # Fold gather+per-node-linear into one dense [STATE_DIM, N_NODES] matrix.
    amat = np.zeros((STATE_DIM, N_NODES), dtype=np.float32)
    cols = np.broadcast_to(np.arange(N_NODES, dtype=np.int64)[:, None], idx.shape)
    np.add.at(amat, (idx.astype(np.int64), cols), W)

    def to_chunks(m):  # [STATE_DIM, X] f32 -> [3, 128, X] f16 (zero padded)
        pad = np.zeros((3 * 128, m.shape[1]), dtype=np.float16)
        pad[:STATE_DIM] = m.astype(np.float16)
        return pad.reshape(3, 128, m.shape[1])

    a3 = to_chunks(amat)  # [3,128,256]
    wfT = np.ascontiguousarray(Wf.T.astype(np.float16))  # [256, 4096]
    # pk2 cols: [k0h0 | k1h0 | k0h1 | k1h1], each [128, 2048]
    pk2 = np.concatenate(
        [wfT[k * 128 : (k + 1) * 128, h * HB : (h + 1) * HB]
         for h in range(2) for k in range(2)],
        axis=1,
    )
    pk2 = np.ascontiguousarray(pk2)
    # 2*b: the tanh is computed as 2*sigmoid(2x + 2b) - 1 on device
    bias2 = np.ascontiguousarray((2.0 * b).reshape(2, 128).T.astype(np.float32))  # [128,2]

    stateT = state.T.astype(np.float16)  # [STATE_DIM, BATCH]
    in_maps = []
    for i in range(N_CORES):
        s3 = to_chunks(stateT[:, i * B_CORE : (i + 1) * B_CORE])  # [3,128,1024]
        pk1 = np.concatenate(
            [s3[0], a3[0], s3[1], a3[1], s3[2], a3[2]], axis=1
        )  # [128, 3840], grouped per c-chunk so each chunk is one DMA
        in_maps.append(
            {
                "pk1": np.ascontiguousarray(pk1),
                "pk2": pk2,
                "bvec": bias2,
            }
        )
    return in_maps


def _run(inputs: dict, trace: bool = False):
    nc = _build_program()
    in_maps = _prepare_in_maps(**inputs)
    res = run_bass_kernel_spmd(
        nc, in_maps, list(range(N_CORES)), trace=trace
    )
    out = np.concatenate([res.results[i]["out"] for i in range(N_CORES)], axis=0)
    return out, res


def kernel(**inputs) -> np.ndarray:
    out, _ = _run(inputs, trace=False)
    return out


if __name__ == "__main__":
    rng = np.random.default_rng(0)
    demo = {
        "state": rng.standard_normal((BATCH, STATE_DIM), dtype=np.float32),
        "W": rng.standard_normal((N_NODES, 27), dtype=np.float32),
        "b": rng.standard_normal(N_NODES, dtype=np.float32),
        "Wf": rng.standard_normal((ACTION, N_NODES), dtype=np.float32),
        "idx": rng.integers(0, STATE_DIM, size=(N_NODES, 27)).astype(np.int32),
    }
    o = kernel(**demo)
    print(o.shape, o.dtype)


# revision 19
# speedup vs baseline: 1.0618x; 1.0618x over previous
"""Trainium2 Bass kernel for MultiInputModel (gnn_message_passing).

Math:
    gathered = state[:, idx]                       # [B, N, E]
    y   = tanh(einsum('bne,ne->bn', gathered, W) + b)   # [B, N]
    out = 500 * sigmoid(y @ Wf.T)                  # [B, A]

The gather + per-node linear is folded on the host into one dense matrix
A[c, n] = sum_e W[n, e] * [idx[n, e] == c], so the device computes two dense
matmuls with fused activations:
    yT  = tanh(A.T @ stateT + b)        # [N, Bc]  (node dim on partitions)
    out = 500 * sigmoid(yT.T @ WfT)     # [Bc, A]  (batch dim on partitions)

Matmul operands are fp16 (1 PE cycle/row vs 4 for fp32; half the input DMA
bytes); accumulation is fp32 in PSUM and the activations/output stay fp32.

Sharding: batch 8192 -> 8 cores x 1024 rows; A / b / WfT replicated.

Input layout: everything the matmuls read is packed on the host into two
128-partition fp16 DRAM tensors so the whole input side is 3 large DMAs:
  pk1 [128, 3840]: stateT as 3 c-chunks [128,1024] + A as 3 c-chunks [128,256]
  pk2 [128, 8192]: WfT [256,4096] as [k0h0|k1h0|k0h1|k1h1] 2048-col groups
                   (h = 2048-wide half of the action dim), DMA'd per half.
"""

import numpy as np

import concourse.bass as bass
import concourse.tile as tile
from concourse import bacc, mybir
from concourse.bass_utils import run_bass_kernel_spmd

N_CORES = 8
BATCH = 8192
B_CORE = BATCH // N_CORES  # 1024
STATE_DIM = 322
N_NODES = 256
ACTION = 4096

F32 = mybir.dt.float32
F16 = mybir.dt.float16
# contraction (state-dim) chunks: 322 = 128 + 128 + 66
C_CHUNKS = [(0, 128), (128, 128), (256, 66)]
AF = mybir.ActivationFunctionType

HB = 2048  # half-block: ACT/DVE/psum granularity (4 PSUM banks)
PK1_COLS = 3 * B_CORE + 3 * N_NODES  # 3840
PK2_COLS = 2 * ACTION  # 8192


def _build_program() -> bass.Bass:
    # Bacc (not raw Bass): its compile pipeline splits multi-sem waits
    # (move_matmul_waits_to_ldweights / generate_event_semaphores) that the
    # TRN2 ISA requires — raw Bass programs fail walrus codegen on any
    # matmul with >1 semaphore wait.
    nc = bacc.Bacc("TRN2", target_bir_lowering=False, debug=False,
                   num_devices=N_CORES)

    pk1 = nc.dram_tensor("pk1", [128, PK1_COLS], F16, kind="ExternalInput")
    pk2 = nc.dram_tensor("pk2", [128, PK2_COLS], F16, kind="ExternalInput")
    bvec = nc.dram_tensor("bvec", [128, 2], F32, kind="ExternalInput")
    out = nc.dram_tensor("out", [B_CORE, ACTION], F32, kind="ExternalOutput")

    with tile.TileContext(nc) as tc:
        with (
            tc.tile_pool(name="persist", bufs=1) as pp,
            tc.tile_pool(name="sig", bufs=3) as sigp,
            tc.tile_pool(name="obuf", bufs=4) as op,
            tc.tile_pool(name="ps", bufs=2, space="PSUM") as pso,
        ):
            # warm the sigmoid ACT table while input DMAs stream: every
            # activation below is Sigmoid (tanh(x) = 2*sigmoid(2x) - 1), so
            # the one table load happens here, off the critical path.
            warm = pp.tile([128, 1], F32, tag="warm")
            nc.vector.memset(warm, 0.0)
            nc.scalar.activation(out=warm, in_=warm, func=AF.Sigmoid)

            # ---- input DMAs: bias, phase-A pack per c-chunk, 2x wf halves ----
            CGRP = B_CORE + N_NODES  # one c-chunk group: stateT cols + A cols
            t1 = pp.tile([128, PK1_COLS], F16, tag="t1")
            bias_t = pp.tile([128, 2], F32, tag="bias")
            for ci in range(3):
                nc.sync.dma_start(
                    out=t1[:, ci * CGRP : (ci + 1) * CGRP],
                    in_=pk1[:, ci * CGRP : (ci + 1) * CGRP],
                )
                if ci == 0:
                    nc.sync.dma_start(out=bias_t, in_=bvec[:, :])
            t2 = pp.tile([128, PK2_COLS], F16, tag="t2")
            for h in range(2):
                nc.sync.dma_start(
                    out=t2[:, h * ACTION : (h + 1) * ACTION],
                    in_=pk2[:, h * ACTION : (h + 1) * ACTION],
                )

            def s_ap(ci, bsl):  # stateT chunk ci, batch slice
                return t1[:, ci * CGRP : ci * CGRP + B_CORE][:, bsl]

            def a_ap(ci, nsl):  # A chunk ci, node slice
                base = ci * CGRP + B_CORE
                return t1[:, base : base + N_NODES][:, nsl]

            def wf_ap(k, ai):  # WfT k-half, 512-wide action chunk ai
                h, aj = divmod(ai, HB // 512)
                base = h * ACTION + k * HB
                return t2[:, base + aj * 512 : base + (aj + 1) * 512]

            y_sb = [
                pp.tile([128, B_CORE], F16, tag=f"y{k}", name=f"y{k}")
                for k in range(2)
            ]

            # ---- phase A: yT = tanh(A.T @ stateT + b)  [256, B_CORE] ----
            # tanh(x) = 2*sigmoid(2x) - 1: ACT does sigmoid(2x + 2b) (host
            # passes 2b), DVE applies the 2t-1 affine while casting to f16.
            # bj-outer so the low batch columns (which phase B reads first)
            # finish first.
            ps = pso.tile([128, HB], F32, tag="ps", name="ps_a")
            for bj in range(B_CORE // 512):
                for nk in range(2):
                    dst = ps[:, nk * 1024 + bj * 512 : nk * 1024 + (bj + 1) * 512]
                    bsl = slice(bj * 512, (bj + 1) * 512)
                    nsl = slice(nk * 128, (nk + 1) * 128)
                    for ci, (c0, cl) in enumerate(C_CHUNKS):
                        nc.tensor.matmul(
                            dst,
                            lhsT=a_ap(ci, nsl)[:cl],
                            rhs=s_ap(ci, bsl)[:cl],
                            start=(ci == 0),
                            stop=(ci == len(C_CHUNKS) - 1),
                        )
            for bj in range(B_CORE // 512):
                for nk in range(2):
                    t = sigp.tile([128, 512], F32, tag=f"sa{nk}{bj}",
                                  name=f"sa{nk}{bj}")
                    nc.scalar.activation(
                        out=t,
                        in_=ps[:, nk * 1024 + bj * 512 : nk * 1024 + (bj + 1) * 512],
                        func=AF.Sigmoid,
                        bias=bias_t[:, nk : nk + 1],
                        scale=2.0,
                    )
                    nc.vector.tensor_scalar(
                        out=y_sb[nk][:, bj * 512 : (bj + 1) * 512],
                        in0=t,
                        scalar1=2.0,
                        scalar2=-1.0,
                        op0=mybir.AluOpType.mult,
                        op1=mybir.AluOpType.add,
                    )

            # ---- phase B: out = 500 * sigmoid(yT.T @ WfT)  [B_CORE, A] ----
            for bi in range(B_CORE // 128):
                ot = op.tile([128, ACTION], F32, tag="ot")
                for h in range(ACTION // HB):
                    ps = pso.tile([128, HB], F32, tag="ps")
                    for aj in range(HB // 512):
                        ai = h * (HB // 512) + aj
                        for k in range(2):
                            nc.tensor.matmul(
                                ps[:, aj * 512 : (aj + 1) * 512],
                                lhsT=y_sb[k][:, bi * 128 : (bi + 1) * 128],
                                rhs=wf_ap(k, ai),
                                start=(k == 0),
                                stop=(k == 1),
                            )
                    sg = sigp.tile([128, HB], F32, tag="sg")
                    nc.scalar.activation(out=sg, in_=ps, func=AF.Sigmoid)
                    nc.vector.tensor_scalar_mul(
                        ot[:, h * HB : (h + 1) * HB], sg, 500.0
                    )
                    # alternate output DMAs across the two DGE paths (SP
                    # HWDGE ring / GpSimd SWDGE ring) so the drain spreads
                    # over more SDMA engine slots and the tail doesn't
                    # serialize on one engine.
                    dma_eng = nc.sync if (bi * 2 + h) % 2 == 0 else nc.gpsimd
                    dma_eng.dma_start(
                        out=out[bi * 128 : (bi + 1) * 128, h * HB : (h + 1) * HB],
                        in_=ot[:, h * HB : (h + 1) * HB],
                    )

    nc.finalize()  # Bacc.finalize -> compile(): reg alloc, wait splitting, ...
    return nc


def _prepare_in_maps(state, W, b, Wf, idx):
    state = np.asarray(state, dtype=np.float32)
    W = np.asarray(W, dtype=np.float32)
    b = np.asarray(b, dtype=np.float32)
    Wf = np.asarray(Wf, dtype=np.float32)
    idx = np.asarray(idx)

    # Fold gather+per-node-linear into one dense [STATE_DIM, N_NODES] matrix.
    amat = np.zeros((STATE_DIM, N_NODES), dtype=np.float32)
    cols = np.broadcast_to(np.arange(N_NODES, dtype=np.int64)[:, None], idx.shape)
    np.add.at(amat, (idx.astype(np.int64), cols), W)

    def to_chunks(m):  # [STATE_DIM, X] f32 -> [3, 128, X] f16 (zero padded)
        pad = np.zeros((3 * 128, m.shape[1]), dtype=np.float16)
        pad[:STATE_DIM] = m.astype(np.float16)
        return pad.reshape(3, 128, m.shape[1])

    a3 = to_chunks(amat)  # [3,128,256]
    wfT = np.ascontiguousarray(Wf.T.astype(np.float16))  # [256, 4096]
    # pk2 cols: [k0h0 | k1h0 | k0h1 | k1h1], each [128, 2048]
    pk2 = np.concatenate(
        [wfT[k * 128 : (k + 1) * 128, h * HB : (h + 1) * HB]
         for h in range(2) for k in range(2)],
        axis=1,
    )
    pk2 = np.ascontiguousarray(pk2)
    # 2*b: the tanh is computed as 2*sigmoid(2x + 2b) - 1 on device
    bias2 = np.ascontiguousarray((2.0 * b).reshape(2, 128).T.astype(np.float32))  # [128,2]

    stateT = state.T.astype(np.float16)  # [STATE_DIM, BATCH]
    in_maps = []
    for i in range(N_CORES):
        s3 = to_chunks(stateT[:, i * B_CORE : (i + 1) * B_CORE])  # [3,128,1024]
        pk1 = np.concatenate(
            [s3[0], a3[0], s3[1], a3[1], s3[2], a3[2]], axis=1
        )  # [128, 3840], grouped per c-chunk so each chunk is one DMA
        in_maps.append(
            {
                "pk1": np.ascontiguousarray(pk1),
                "pk2": pk2,
                "bvec": bias2,
            }
        )
    return in_maps


def _run(inputs: dict, trace: bool = False):
    nc = _build_program()
    in_maps = _prepare_in_maps(**inputs)
    res = run_bass_kernel_spmd(
        nc, in_maps, list(range(N_CORES)), trace=trace
    )
    out = np.concatenate([res.results[i]["out"] for i in range(N_CORES)], axis=0)
    return out, res


def kernel(**inputs) -> np.ndarray:
    out, _ = _run(inputs, trace=False)
    return out


if __name__ == "__main__":
    rng = np.random.default_rng(0)
    demo = {
        "state": rng.standard_normal((BATCH, STATE_DIM), dtype=np.float32),
        "W": rng.standard_normal((N_NODES, 27), dtype=np.float32),
        "b": rng.standard_normal(N_NODES, dtype=np.float32),
        "Wf": rng.standard_normal((ACTION, N_NODES), dtype=np.float32),
        "idx": rng.integers(0, STATE_DIM, size=(N_NODES, 27)).astype(np.int32),
    }
    o = kernel(**demo)
    print(o.shape, o.dtype)
